# revision 28
# baseline (speedup 1.0000x reference)
"""Bass/Tile kernel for chunkwise retention (nn_ChunkwiseRetention).

Algorithm (per core = one batch element, seq 4000, B=5, 800 chunks):
superchunks of G=25 chunks (125 positions). The host pre-scales
xqT columns by g6^j and xkT by g6^-j (j = global chunk index), which
folds the entire cross-chunk decay into the projections: the cross
mask becomes 0/1, the carry is Q~ @ U with no rescale, and the state
update needs no scaling at all.

Everything on the PE runs in bf16 (1 cycle/row at any moving size in
the cost model, vs f32r's >=256-even constraint); PSUM accumulation
stays f32. Inputs are pre-cast to bf16 on the host (halves DMA bytes)
and packed into ONE dram tensor so each 4-superchunk group is a single
DMA (HWDGE is a serial ~625ns/DMA resource). Output walls are written
in pairs per DMA for the same reason.

Per superchunk s: Q~^T/K~^T projected per PAIR of superchunks (N=250)
into single-bank PSUM tiles (tag-rotated, bufs=2, so the ACT copy of
one pair overlaps the projection of the next); K~,V (pos-major)
projections; P~^T = K~ @ Q~^T (bf16, N=125); masked matmuls accumulate
cross + intra (+5-row shift via free-dim-shifted stationary) + seam
(previous superchunk's tail stationary x previous V) + carry (Q~ @ U)
into one PSUM window; running state U in one PSUM bank (zero-matmul
init, per-element has_written accumulation).

PSUM banks (8): qk 2x1 + kv 2 + pt 1 + wt 2 + u 1.
"""
import ml_dtypes
import numpy as np

import concourse.bass as bass
import concourse.mybir as mybir
import concourse.tile as tile

GAMMA = 0.9865
B = 5
SEQ = 4000
FEAT = 256
DIM = 256
G = 25
GP = G * B            # 125
NSC = SEQ // GP       # 32
NPAIR = NSC // 2      # 16
LG = 4                # superchunks per DMA load group
LGP = LG * GP         # 500
F32 = mybir.dt.float32
F32R = mybir.dt.float32r
BF16 = mybir.dt.bfloat16
NP_BF16 = ml_dtypes.bfloat16
g6 = float(np.float64(GAMMA) ** 6)
COPY = mybir.ActivationFunctionType.Copy

# const blob column layout
C_WCT = 0            # [0:125)   0/1 strict lower-block-triangular cross mask
C_WIT = 125          # [125:250) intra decay mask (rows 0:125)
C_Z = 250            # [250:762) zeros (row 0 used as zero matmul operand)
C_END = 762


def make_const_blob():
    t = np.arange(GP) // B
    p = np.arange(GP) % B
    tb, ta = t[:, None], t[None, :]
    wct01 = (tb < ta).astype(np.float32)
    qb, pa = p[:, None], p[None, :]
    wit = np.where((tb == ta) & (pa >= qb),
                   np.float64(GAMMA) ** (qb - pa), 0.0).astype(np.float32)
    blob = np.zeros((128, C_END), np.float32)
    blob[0:GP, C_WCT:C_WCT + 125] = wct01
    blob[0:GP, C_WIT:C_WIT + 125] = wit
    return blob


def build_kernel(nc: bass.Bass):
    xin = nc.dram_tensor("xin", [3, FEAT, SEQ], BF16, kind="ExternalInput").ap()
    wqkv = nc.dram_tensor("wqkv", [FEAT, 3 * DIM + 128], BF16, kind="ExternalInput").ap()
    out = nc.dram_tensor("out", [SEQ, DIM], BF16, kind="ExternalOutput").ap()

    blob_np = make_const_blob()
    mm = nc.tensor.matmul

    with tile.TileContext(nc) as tc:
        with (
            tc.tile_pool(name="consts", bufs=1) as cpool,
            tc.tile_pool(name="xin", bufs=3) as xpool,
            tc.tile_pool(name="work", bufs=2) as spool,
            tc.tile_pool(name="psQK", bufs=2, space="PSUM") as psQK,
            tc.tile_pool(name="psP", bufs=3, space="PSUM") as psP,
            tc.tile_pool(name="psW", bufs=2, space="PSUM") as psW,
            tc.tile_pool(name="psU", bufs=1, space="PSUM") as psU,
        ):
            # --- constants to SBUF. Order matters at startup: weights
            # first (first projections need them), then the first
            # half-group of x, then the mask blob (first needed by the
            # DVE muls, ~2us in). ---
            w_sb = cpool.tile([128, 2, 3 * DIM + 128], BF16, name="w_sb")
            wk_sb = w_sb[:, :, 256:512]
            wv_sb = w_sb[:, :, 512:768]
            ident_sb = w_sb[:, 0, 768:896]
            blob_sb = cpool.tile([128, C_END], F32R, name="blob_sb")
            wct_sb = blob_sb[0:GP, C_WCT:C_WCT + 125]
            wit_sb = blob_sb[0:GP, C_WIT:C_WIT + 125]

            u_ps = psU.tile([128, 512], F32, name="u_state")


            # persistent combined mask stationaries (manual triple-buffer):
            # cols 0:125 = mpc (cross), cols 125:375 = mpi buffer (window
            # slice 125:250 = 5-shifted intra, seam slice 250:375). The
            # fused DVE mul writes cols {0:125} u {130:255}; zero columns
            # are memset once.
            cmb_bufs = []
            for i_ in range(3):
                mb_ = spool.tile([125, 380], BF16, name=f"cmb_{i_}",
                                 tag=f"cmb_{i_}", bufs=1)
                nc.vector.memset(mb_[:, 125:130], 0.0)
                nc.vector.memset(mb_[:, 255:380], 0.0)
                cmb_bufs.append(mb_)

            prev_cmb = prev_v = None
            wtpt = {}
            ut_tile = {}
            xg_tiles = {}
            pair_sb = {}

            def load_group(g, split=False):
                if split:
                    # group 0 is a padded 512-col tile loaded as two 256-col
                    # DMAs: 256 bf16 cols = 512B runs, the minimum for
                    # full-rate DMA (sub-512B runs pay 2x). Order: first
                    # half of x, rest of weights, masks, second half of x.
                    t = cpool.tile([128, 3, 2, 512], BF16, name="x_0")
                    nc.sync.dma_start(
                        out=t[:, :, :, 0:256],
                        in_=xin[:, :, 0:256].rearrange(
                            "t (h p) a -> p t h a", p=128))
                    nc.sync.dma_start(
                        out=w_sb[:, :, 0:256],
                        in_=wqkv[:, 0:256].rearrange("(h p) d -> p h d",
                                                     p=128))
                    nc.sync.dma_start(
                        out=w_sb[:, :, 256:896],
                        in_=wqkv[:, 256:896].rearrange("(h p) d -> p h d",
                                                       p=128))
                    nc.gpsimd.dma_start(out=blob_sb,
                                        in_=nc.inline_tensor(blob_np, "cblob")
                                        .ap().bitcast(F32R))
                    nc.gpsimd.dma_start(
                        out=t[:, :, :, 256:512],
                        in_=xin[:, :, 256:512].rearrange(
                            "t (h p) a -> p t h a", p=128))
                else:
                    t = xpool.tile([128, 3, 2, LGP], BF16, name=f"x_{g}",
                                   tag="x")
                    nc.sync.dma_start(
                        out=t,
                        in_=xin[:, :, g * LGP:(g + 1) * LGP].rearrange(
                            "t (h p) a -> p t h a", p=128))
                xg_tiles[g] = t

            def proj_pair(p):
                """Q~^T and K~^T (dim-major) for superchunks 2p, 2p+1."""
                g, pl = divmod(p, 2)
                xg = xg_tiles[g]
                csl = slice(pl * 2 * GP, (pl * 2 + 2) * GP)   # 250 cols
                qt2 = spool.tile([128, 2, 250], BF16, name=f"qt_{p}", tag="qt")
                kt2 = spool.tile([128, 2, 250], BF16, name=f"kt_{p}", tag="kt")
                for tidx, wlo, dst in ((0, 0, qt2), (1, 256, kt2)):
                    ps = psQK.tile([128, 2, 256], F32, name=f"qk_{tidx}_{p}",
                                   tag="qkps")
                    for dh in (0, 1):
                        for h in (0, 1):
                            mm(ps[:, dh, 0:250],
                               w_sb[:, h, wlo + dh * 128:wlo + dh * 128 + 128],
                               xg[:, tidx, h, csl],
                               start=(h == 0), stop=(h == 1))
                    nc.scalar.activation(dst, ps[:, :, 0:250], COPY)
                pair_sb[p] = (qt2, kt2)
                if p >= 2:
                    pair_sb.pop(p - 2, None)

            def prep_pt(s):
                """P~^T + masked stationaries for superchunk s (emitted an
                iteration early, so the DVE muls overlap the previous
                window)."""
                qt2, kt2 = pair_sb[s // 2]
                m = s % 2
                qlo = qt2[:, 0, m * GP:(m + 1) * GP]
                qhi = qt2[:, 1, m * GP:(m + 1) * GP]
                klo = kt2[:, 0, m * GP:(m + 1) * GP]
                khi = kt2[:, 1, m * GP:(m + 1) * GP]

                # P~^T = K~ @ Q~^T (bf16: 1 cycle/row at N=125) into the
                # spare half of THIS superchunk's window bank (allocated here,
                # one iteration ahead of the window matmuls)
                wtp = psW.tile([125, 512], F32, name=f"wt_{s}", tag="wt")
                wtpt[s] = wtp
                pt_ps = wtp[:, 256:381]
                mm(pt_ps, klo, qlo, start=True, stop=False)
                mm(pt_ps, khi, qhi, start=False, stop=True)

                cmb = cmb_bufs[s % 3]

                def emit_mul():
                    # one fused mul: out cols {0:125, 130:255}, pt read twice
                    nc.vector.tensor_mul(
                        cmb[:, 0:260].rearrange("p (b c) -> p b c", c=130)[:, :, 0:125],
                        pt_ps.unsqueeze(1).broadcast_to([125, 2, 125]),
                        blob_sb[0:GP, 0:250].rearrange("p (b c) -> p b c", c=125))
                return dict(cmb=cmb, qlo=qlo, qhi=qhi, emit_mul=emit_mul)

            def prep_kv(s):
                """K~/V pos-major for superchunk s + prefetches."""
                g = s // LG
                xg = xg_tiles[g]
                lsl = slice((s % LG) * GP, (s % LG + 1) * GP)
                qt2, kt2 = pair_sb[s // 2]
                m = s % 2
                klo = kt2[:, 0, m * GP:(m + 1) * GP]
                khi = kt2[:, 1, m * GP:(m + 1) * GP]
                kv = psP.tile([125, 512], F32, name=f"kv_{s}", tag="kv")
                nc.tensor.transpose(
                    kv[:, 0:64].bitcast(BF16), klo, ident_sb)
                nc.tensor.transpose(
                    kv[:, 64:128].bitcast(BF16), khi, ident_sb)
                for h in (0, 1):
                    mm(kv[:, 256:512], xg[:, 2, h, lsl], wv_sb[:, h, :],
                       start=(h == 0), stop=(h == 1))
                kv_sb = spool.tile([125, 512], BF16, name=f"kv_sb_{s}",
                                   tag="kvsb", bufs=5)
                k_sb = kv_sb[:, 0:256]
                v_sb = kv_sb[:, 256:512]
                nc.vector.tensor_copy(k_sb, kv[:, 0:128].bitcast(BF16))
                nc.vector.tensor_copy(v_sb, kv[:, 256:512])
                return dict(k_sb=k_sb, v_sb=v_sb)

            def prefetch(s):
                # next x group at group boundaries, next qk pair two pairs
                # ahead; emitted AFTER the ut snapshot so the ACT queue
                # serves ut (on the state->carry recurrence) first
                if s % LG == 0 and s // LG + 2 < SEQ // LGP:
                    load_group(s // LG + 2)
                if s % 2 == 1 and s // 2 + 2 < NPAIR:
                    proj_pair(s // 2 + 2)

            # prologue
            load_group(0, split=True)
            load_group(1)
            # zero-matmul initializes the U bank's data + has_written bits so
            # the per-superchunk state matmuls can all accumulate (emitted
            # after the blob DMA so the zeros dependency is tracked)
            mm(u_ps, blob_sb[0:1, C_Z:C_Z + 128], blob_sb[0:1, C_Z:C_Z + 512],
               start=True, stop=True, skip_group_check=True)
            proj_pair(0)
            proj_pair(1)
            stp = prep_pt(0)
            stk = prep_kv(0)
            stp["emit_mul"]()
            prefetch(0)
            wall_pair = None

            for s in range(NSC):
                ut_sb_cur = ut_tile.get("t")
                k_sb, v_sb = stk["k_sb"], stk["v_sb"]
                cmb = stp["cmb"]
                qlo, qhi = stp["qlo"], stp["qhi"]

                # this window's PSUM bank was allocated by prep_pt(s) an
                # iteration ago (wt cols 0:256, its PT came in cols 256:381).
                # Emit PT+masks for s+1 FIRST: the DVE muls then overlap this
                # window's matmuls.
                wtp = wtpt.pop(s)
                wt = wtp[:, 0:256]
                if s + 1 < NSC:
                    stp = prep_pt(s + 1)
                    stk_next = prep_kv(s + 1)
                    stp["emit_mul"]()

                if s == NSC - 1:
                    # final output chunk 799 = intra tail of the last
                    # superchunk; emitted before the last window so the
                    # kernel tail isn't serialized behind it
                    wtf = wtp[:, 256:512]
                    mm(wtf, cmb[:, 250:375], v_sb, start=True, stop=True)
                    wallf_sb = spool.tile([5, 256], BF16, name="wallf",
                                          tag="wallf")
                    nc.vector.tensor_copy(wallf_sb, wtf[0:5, 0:256])
                    nc.sync.dma_start(out=out[SEQ - B:SEQ], in_=wallf_sb)

                # --- window accumulation (one closed group). The seam
                # (intra tail of chunk s*G-1) is added directly from the
                # previous superchunk's tail stationary and V: rows 5:125 of
                # that matmul multiply zero columns and accumulate zeros. ---
                mm(wt, cmb[:, 0:125], v_sb, start=True, stop=False)
                mm(wt, cmb[:, 125:250], v_sb, start=False, stop=(s == 0))
                if s > 0:
                    mm(wt, prev_cmb[:, 250:375], prev_v, start=False, stop=False)
                    mm(wt, qlo, ut_sb_cur[:, 0:256], start=False, stop=False)
                    mm(wt, qhi, ut_sb_cur[:, 256:512], start=False, stop=True)

                # --- state update (accumulates; U bank bits set by zero-mm) ---
                mm(u_ps[:, 0:256], k_sb[:, 0:128], v_sb,
                   start=False, stop=True, skip_group_check=True)
                mm(u_ps[:, 256:512], k_sb[:, 128:256], v_sb,
                   start=False, stop=True, skip_group_check=True)
                # U snapshot for iteration s+1's carry, emitted immediately so
                # it heads the ACT queue (it's on the state->carry recurrence)
                if s + 1 < NSC:
                    utn = spool.tile([128, 512], BF16, name=f"ut_{s + 1}",
                                     tag="ut", bufs=3)
                    nc.scalar.activation(utn, u_ps, COPY)
                    ut_tile["t"] = utn
                    prefetch(s + 1)

                if s + 1 < NSC:
                    stk = stk_next

                # --- output (paired DMAs: (1,2),(3,4),...,(29,30); 0 and 31
                # single). HWDGE is a serial per-DMA overhead, so fewer DMAs. ---
                if s == 0:
                    w0 = spool.tile([125, 2, 256], BF16, name="wall_0",
                                    tag="wall", bufs=3)
                    nc.vector.tensor_copy(w0[:, 0, :], wt)
                    nc.sync.dma_start(out=out[0:GP - B], in_=w0[B:GP, 0, :])
                elif s == NSC - 1:
                    wl = spool.tile([125, 2, 256], BF16, name=f"wall_{s}",
                                    tag="wall", bufs=3)
                    nc.scalar.activation(wl[:, 0, :], wt, COPY)
                    nc.sync.dma_start(out=out[s * GP - B: s * GP - B + GP],
                                      in_=wl[:, 0, :])
                elif s % 2 == 1:
                    wall_pair = spool.tile([125, 2, 256], BF16, name=f"wall_{s}",
                                           tag="wall", bufs=3)
                    nc.vector.tensor_copy(wall_pair[:, 0, :], wt)
                else:
                    nc.vector.tensor_copy(wall_pair[:, 1, :], wt)
                    base = (s - 1) * GP - B
                    nc.sync.dma_start(
                        out=out[base:base + 2 * GP].rearrange(
                            "(b p) d -> p b d", b=2),
                        in_=wall_pair)
                prev_cmb, prev_v = cmb, v_sb

    return nc


def _col_scales():
    j = np.arange(SEQ) // B          # global chunk index
    sq = (np.float64(g6) ** j).astype(np.float32)
    sk = (np.float64(g6) ** (-j)).astype(np.float32)
    return sq, sk


def prep_core_inputs(xq2d, xk2d, xv2d, wqkv):
    sq, sk = _col_scales()
    xin = np.stack([
        (xq2d.T * sq[None, :]).astype(NP_BF16),
        (xk2d.T * sk[None, :]).astype(NP_BF16),
        xv2d.T.astype(NP_BF16),
    ], axis=0)
    return {
        "xin": np.ascontiguousarray(xin),
        "wqkv": wqkv.astype(NP_BF16),
    }


def make_in_maps(inputs):
    """inputs: dict from setup_inputs (full batch). Returns per-core in_maps."""
    xq, xk, xv = inputs["xq"], inputs["xk"], inputs["xv"]
    ident = np.zeros((256, 128), np.float32)
    ident[0:128, 0:128] = np.eye(128, dtype=np.float32)
    wqkv = np.ascontiguousarray(np.concatenate(
        [np.asarray(inputs["Wq"], dtype=np.float32),
         np.asarray(inputs["Wk"], dtype=np.float32),
         np.asarray(inputs["Wv"], dtype=np.float32), ident], axis=1))
    in_maps = []
    for b in range(8):
        in_maps.append(prep_core_inputs(
            np.asarray(xq[b], dtype=np.float32),
            np.asarray(xk[b], dtype=np.float32),
            np.asarray(xv[b], dtype=np.float32), wqkv))
    return in_maps


_NC_CACHE = {}


def _get_nc():
    if "nc" not in _NC_CACHE:
        from concourse import bacc
        nc = bacc.Bacc("TRN2", target_bir_lowering=False, debug=False)
        build_kernel(nc)
        nc.compile()
        _NC_CACHE["nc"] = nc
    return _NC_CACHE["nc"]


def run(inputs, trace=False, **kwargs):
    """Run on 8 NeuronCores; returns (output [8,4000,256], BassKernelResults)."""
    from concourse.bass_utils import run_bass_kernel_spmd

    nc = _get_nc()
    in_maps = make_in_maps(inputs)
    res = run_bass_kernel_spmd(nc, in_maps, core_ids=list(range(8)),
                               trace=trace, **kwargs)
    out = np.stack([np.asarray(r["out"]).astype(np.float32)
                    for r in res.results], axis=0)
    return out, res


def kernel(**inputs) -> np.ndarray:
    out, _ = run(inputs)
    return out


# revision 29
# speedup vs baseline: 1.0175x; 1.0175x over previous
"""Bass/Tile kernel for chunkwise retention (nn_ChunkwiseRetention).

Algorithm (per core = one batch element, seq 4000, B=5, 800 chunks):
superchunks of G=25 chunks (125 positions). The host pre-scales
xqT columns by g6^j and xkT by g6^-j (j = global chunk index), which
folds the entire cross-chunk decay into the projections: the cross
mask becomes 0/1, the carry is Q~ @ U with no rescale, and the state
update needs no scaling at all.

Everything on the PE runs in bf16 (1 cycle/row at any moving size in
the cost model, vs f32r's >=256-even constraint); PSUM accumulation
stays f32. Inputs are pre-cast to bf16 on the host (halves DMA bytes)
and packed into ONE dram tensor so each 4-superchunk group is a single
DMA (HWDGE is a serial ~625ns/DMA resource). Output walls are written
in pairs per DMA for the same reason.

Per superchunk s: Q~^T/K~^T projected per PAIR of superchunks (N=250)
into single-bank PSUM tiles (tag-rotated, bufs=2, so the ACT copy of
one pair overlaps the projection of the next); K~,V (pos-major)
projections; P~^T = K~ @ Q~^T (bf16, N=125); masked matmuls accumulate
cross + intra (+5-row shift via free-dim-shifted stationary) + seam
(previous superchunk's tail stationary x previous V) + carry (Q~ @ U)
into one PSUM window; running state U in one PSUM bank (zero-matmul
init, per-element has_written accumulation).

PSUM banks (8): qk 2x1 + kv 2 + pt 1 + wt 2 + u 1.
"""
import ml_dtypes
import numpy as np

import concourse.bass as bass
import concourse.mybir as mybir
import concourse.tile as tile

GAMMA = 0.9865
B = 5
SEQ = 4000
FEAT = 256
DIM = 256
G = 25
GP = G * B            # 125
NSC = SEQ // GP       # 32
NPAIR = NSC // 2      # 16
LG = 4                # superchunks per DMA load group
LGP = LG * GP         # 500
F32 = mybir.dt.float32
F32R = mybir.dt.float32r
BF16 = mybir.dt.bfloat16
NP_BF16 = ml_dtypes.bfloat16
g6 = float(np.float64(GAMMA) ** 6)
COPY = mybir.ActivationFunctionType.Copy

# const blob column layout
C_WCT = 0            # [0:125)   0/1 strict lower-block-triangular cross mask
C_WIT = 125          # [125:250) intra decay mask (rows 0:125)
C_Z = 250            # [250:762) zeros (row 0 used as zero matmul operand)
C_END = 762


def make_const_blob():
    t = np.arange(GP) // B
    p = np.arange(GP) % B
    tb, ta = t[:, None], t[None, :]
    wct01 = (tb < ta).astype(np.float32)
    qb, pa = p[:, None], p[None, :]
    wit = np.where((tb == ta) & (pa >= qb),
                   np.float64(GAMMA) ** (qb - pa), 0.0).astype(np.float32)
    blob = np.zeros((128, C_END), np.float32)
    blob[0:GP, C_WCT:C_WCT + 125] = wct01
    blob[0:GP, C_WIT:C_WIT + 125] = wit
    return blob


def build_kernel(nc: bass.Bass):
    xin = nc.dram_tensor("xin", [3, FEAT, SEQ], BF16, kind="ExternalInput").ap()
    wqkv = nc.dram_tensor("wqkv", [FEAT, 3 * DIM + 128], BF16, kind="ExternalInput").ap()
    out = nc.dram_tensor("out", [SEQ, DIM], BF16, kind="ExternalOutput").ap()

    blob_np = make_const_blob()
    mm = nc.tensor.matmul

    with tile.TileContext(nc) as tc:
        with (
            tc.tile_pool(name="consts", bufs=1) as cpool,
            tc.tile_pool(name="xin", bufs=3) as xpool,
            tc.tile_pool(name="work", bufs=2) as spool,
            tc.tile_pool(name="psQK", bufs=2, space="PSUM") as psQK,
            tc.tile_pool(name="psP", bufs=3, space="PSUM") as psP,
            tc.tile_pool(name="psW", bufs=2, space="PSUM") as psW,
            tc.tile_pool(name="psU", bufs=1, space="PSUM") as psU,
        ):
            # --- constants to SBUF. Order matters at startup: weights
            # first (first projections need them), then the first
            # half-group of x, then the mask blob (first needed by the
            # DVE muls, ~2us in). ---
            w_sb = cpool.tile([128, 2, 3 * DIM + 128], BF16, name="w_sb")
            wk_sb = w_sb[:, :, 256:512]
            wv_sb = w_sb[:, :, 512:768]
            ident_sb = w_sb[:, 0, 768:896]
            blob_sb = cpool.tile([128, C_END], F32R, name="blob_sb")
            wct_sb = blob_sb[0:GP, C_WCT:C_WCT + 125]
            wit_sb = blob_sb[0:GP, C_WIT:C_WIT + 125]

            u_ps = psU.tile([128, 512], F32, name="u_state")


            # persistent combined mask stationaries (manual triple-buffer):
            # cols 0:125 = mpc (cross), cols 125:375 = mpi buffer (window
            # slice 125:250 = 5-shifted intra, seam slice 250:375). The
            # fused DVE mul writes cols {0:125} u {130:255}; zero columns
            # are memset once.
            cmb_bufs = []
            for i_ in range(3):
                mb_ = spool.tile([125, 380], BF16, name=f"cmb_{i_}",
                                 tag=f"cmb_{i_}", bufs=1)
                nc.vector.memset(mb_[:, 125:130], 0.0)
                nc.vector.memset(mb_[:, 255:380], 0.0)
                cmb_bufs.append(mb_)

            prev_cmb = prev_v = None
            wtpt = {}
            ut_tile = {}
            xg_tiles = {}
            pair_sb = {}

            def load_group(g, split=False):
                if split:
                    # group 0 is a padded 512-col tile loaded as two 256-col
                    # DMAs: 256 bf16 cols = 512B runs, the minimum for
                    # full-rate DMA (sub-512B runs pay 2x). Order: first
                    # half of x, rest of weights, masks, second half of x.
                    t = cpool.tile([128, 3, 2, 512], BF16, name="x_0")
                    nc.sync.dma_start(
                        out=t[:, :, :, 0:256],
                        in_=xin[:, :, 0:256].rearrange(
                            "t (h p) a -> p t h a", p=128))
                    nc.sync.dma_start(
                        out=w_sb[:, :, 0:256],
                        in_=wqkv[:, 0:256].rearrange("(h p) d -> p h d",
                                                     p=128))
                    nc.sync.dma_start(
                        out=w_sb[:, :, 256:896],
                        in_=wqkv[:, 256:896].rearrange("(h p) d -> p h d",
                                                       p=128))
                    nc.sync.dma_start(out=blob_sb,
                                      in_=nc.inline_tensor(blob_np, "cblob")
                                      .ap().bitcast(F32R))
                    nc.sync.dma_start(
                        out=t[:, :, :, 256:512],
                        in_=xin[:, :, 256:512].rearrange(
                            "t (h p) a -> p t h a", p=128))
                else:
                    t = xpool.tile([128, 3, 2, LGP], BF16, name=f"x_{g}",
                                   tag="x")
                    nc.sync.dma_start(
                        out=t,
                        in_=xin[:, :, g * LGP:(g + 1) * LGP].rearrange(
                            "t (h p) a -> p t h a", p=128))
                xg_tiles[g] = t

            def proj_pair(p):
                """Q~^T and K~^T (dim-major) for superchunks 2p, 2p+1."""
                g, pl = divmod(p, 2)
                xg = xg_tiles[g]
                csl = slice(pl * 2 * GP, (pl * 2 + 2) * GP)   # 250 cols
                qt2 = spool.tile([128, 2, 250], BF16, name=f"qt_{p}", tag="qt")
                kt2 = spool.tile([128, 2, 250], BF16, name=f"kt_{p}", tag="kt")
                for tidx, wlo, dst in ((0, 0, qt2), (1, 256, kt2)):
                    ps = psQK.tile([128, 2, 256], F32, name=f"qk_{tidx}_{p}",
                                   tag="qkps")
                    for dh in (0, 1):
                        for h in (0, 1):
                            mm(ps[:, dh, 0:250],
                               w_sb[:, h, wlo + dh * 128:wlo + dh * 128 + 128],
                               xg[:, tidx, h, csl],
                               start=(h == 0), stop=(h == 1))
                    nc.scalar.activation(dst, ps[:, :, 0:250], COPY)
                pair_sb[p] = (qt2, kt2)
                if p >= 2:
                    pair_sb.pop(p - 2, None)

            def prep_pt(s):
                """P~^T + masked stationaries for superchunk s (emitted an
                iteration early, so the DVE muls overlap the previous
                window)."""
                qt2, kt2 = pair_sb[s // 2]
                m = s % 2
                qlo = qt2[:, 0, m * GP:(m + 1) * GP]
                qhi = qt2[:, 1, m * GP:(m + 1) * GP]
                klo = kt2[:, 0, m * GP:(m + 1) * GP]
                khi = kt2[:, 1, m * GP:(m + 1) * GP]

                # P~^T = K~ @ Q~^T (bf16: 1 cycle/row at N=125) into the
                # spare half of THIS superchunk's window bank (allocated here,
                # one iteration ahead of the window matmuls)
                wtp = psW.tile([125, 512], F32, name=f"wt_{s}", tag="wt")
                wtpt[s] = wtp
                pt_ps = wtp[:, 256:381]
                mm(pt_ps, klo, qlo, start=True, stop=False)
                mm(pt_ps, khi, qhi, start=False, stop=True)

                cmb = cmb_bufs[s % 3]

                def emit_mul():
                    # one fused mul: out cols {0:125, 130:255}, pt read twice
                    nc.vector.tensor_mul(
                        cmb[:, 0:260].rearrange("p (b c) -> p b c", c=130)[:, :, 0:125],
                        pt_ps.unsqueeze(1).broadcast_to([125, 2, 125]),
                        blob_sb[0:GP, 0:250].rearrange("p (b c) -> p b c", c=125))
                return dict(cmb=cmb, qlo=qlo, qhi=qhi, emit_mul=emit_mul)

            def prep_kv(s):
                """K~/V pos-major for superchunk s + prefetches."""
                g = s // LG
                xg = xg_tiles[g]
                lsl = slice((s % LG) * GP, (s % LG + 1) * GP)
                qt2, kt2 = pair_sb[s // 2]
                m = s % 2
                klo = kt2[:, 0, m * GP:(m + 1) * GP]
                khi = kt2[:, 1, m * GP:(m + 1) * GP]
                kv = psP.tile([125, 512], F32, name=f"kv_{s}", tag="kv")
                nc.tensor.transpose(
                    kv[:, 0:64].bitcast(BF16), klo, ident_sb)
                nc.tensor.transpose(
                    kv[:, 64:128].bitcast(BF16), khi, ident_sb)
                for h in (0, 1):
                    mm(kv[:, 256:512], xg[:, 2, h, lsl], wv_sb[:, h, :],
                       start=(h == 0), stop=(h == 1))
                kv_sb = spool.tile([125, 512], BF16, name=f"kv_sb_{s}",
                                   tag="kvsb", bufs=5)
                k_sb = kv_sb[:, 0:256]
                v_sb = kv_sb[:, 256:512]
                nc.vector.tensor_copy(k_sb, kv[:, 0:128].bitcast(BF16))
                nc.vector.tensor_copy(v_sb, kv[:, 256:512])
                return dict(k_sb=k_sb, v_sb=v_sb)

            def prefetch(s):
                # next x group at group boundaries, next qk pair two pairs
                # ahead; emitted AFTER the ut snapshot so the ACT queue
                # serves ut (on the state->carry recurrence) first
                if s % LG == 0 and s // LG + 2 < SEQ // LGP:
                    load_group(s // LG + 2)
                if s % 2 == 1 and s // 2 + 2 < NPAIR:
                    proj_pair(s // 2 + 2)

            # prologue
            load_group(0, split=True)
            load_group(1)
            # zero-matmul initializes the U bank's data + has_written bits so
            # the per-superchunk state matmuls can all accumulate (emitted
            # after the blob DMA so the zeros dependency is tracked)
            mm(u_ps, blob_sb[0:1, C_Z:C_Z + 128], blob_sb[0:1, C_Z:C_Z + 512],
               start=True, stop=True, skip_group_check=True)
            proj_pair(0)
            proj_pair(1)
            stp = prep_pt(0)
            stk = prep_kv(0)
            stp["emit_mul"]()
            prefetch(0)
            wall_pair = None

            for s in range(NSC):
                ut_sb_cur = ut_tile.get("t")
                k_sb, v_sb = stk["k_sb"], stk["v_sb"]
                cmb = stp["cmb"]
                qlo, qhi = stp["qlo"], stp["qhi"]

                # this window's PSUM bank was allocated by prep_pt(s) an
                # iteration ago (wt cols 0:256, its PT came in cols 256:381).
                # Emit PT+masks for s+1 FIRST: the DVE muls then overlap this
                # window's matmuls.
                wtp = wtpt.pop(s)
                wt = wtp[:, 0:256]
                if s + 1 < NSC:
                    stp = prep_pt(s + 1)
                    stk_next = prep_kv(s + 1)
                    stp["emit_mul"]()

                if s == NSC - 1:
                    # final output chunk 799 = intra tail of the last
                    # superchunk; emitted before the last window so the
                    # kernel tail isn't serialized behind it
                    wtf = wtp[:, 256:512]
                    mm(wtf, cmb[:, 250:375], v_sb, start=True, stop=True)
                    wallf_sb = spool.tile([5, 256], BF16, name="wallf",
                                          tag="wallf")
                    nc.vector.tensor_copy(wallf_sb, wtf[0:5, 0:256])
                    nc.sync.dma_start(out=out[SEQ - B:SEQ], in_=wallf_sb)

                # --- window accumulation (one closed group). The seam
                # (intra tail of chunk s*G-1) is added directly from the
                # previous superchunk's tail stationary and V: rows 5:125 of
                # that matmul multiply zero columns and accumulate zeros. ---
                mm(wt, cmb[:, 0:125], v_sb, start=True, stop=False)
                mm(wt, cmb[:, 125:250], v_sb, start=False, stop=(s == 0))
                if s > 0:
                    mm(wt, prev_cmb[:, 250:375], prev_v, start=False, stop=False)
                    mm(wt, qlo, ut_sb_cur[:, 0:256], start=False, stop=False)
                    mm(wt, qhi, ut_sb_cur[:, 256:512], start=False, stop=True)

                # --- state update (accumulates; U bank bits set by zero-mm) ---
                mm(u_ps[:, 0:256], k_sb[:, 0:128], v_sb,
                   start=False, stop=True, skip_group_check=True)
                mm(u_ps[:, 256:512], k_sb[:, 128:256], v_sb,
                   start=False, stop=True, skip_group_check=True)
                # U snapshot for iteration s+1's carry, emitted immediately so
                # it heads the ACT queue (it's on the state->carry recurrence)
                if s + 1 < NSC:
                    utn = spool.tile([128, 512], BF16, name=f"ut_{s + 1}",
                                     tag="ut", bufs=3)
                    nc.scalar.activation(utn, u_ps, COPY)
                    ut_tile["t"] = utn
                    prefetch(s + 1)

                if s + 1 < NSC:
                    stk = stk_next

                # --- output (paired DMAs: (1,2),(3,4),...,(29,30); 0 and 31
                # single). HWDGE is a serial per-DMA overhead, so fewer DMAs. ---
                if s == 0:
                    w0 = spool.tile([125, 2, 256], BF16, name="wall_0",
                                    tag="wall", bufs=3)
                    nc.vector.tensor_copy(w0[:, 0, :], wt)
                    nc.sync.dma_start(out=out[0:GP - B], in_=w0[B:GP, 0, :])
                elif s == NSC - 1:
                    wl = spool.tile([125, 2, 256], BF16, name=f"wall_{s}",
                                    tag="wall", bufs=3)
                    nc.scalar.activation(wl[:, 0, :], wt, COPY)
                    nc.sync.dma_start(out=out[s * GP - B: s * GP - B + GP],
                                      in_=wl[:, 0, :])
                elif s % 2 == 1:
                    wall_pair = spool.tile([125, 2, 256], BF16, name=f"wall_{s}",
                                           tag="wall", bufs=3)
                    nc.vector.tensor_copy(wall_pair[:, 0, :], wt)
                else:
                    nc.vector.tensor_copy(wall_pair[:, 1, :], wt)
                    base = (s - 1) * GP - B
                    nc.sync.dma_start(
                        out=out[base:base + 2 * GP].rearrange(
                            "(b p) d -> p b d", b=2),
                        in_=wall_pair)
                prev_cmb, prev_v = cmb, v_sb

    return nc


def _col_scales():
    j = np.arange(SEQ) // B          # global chunk index
    sq = (np.float64(g6) ** j).astype(np.float32)
    sk = (np.float64(g6) ** (-j)).astype(np.float32)
    return sq, sk


def prep_core_inputs(xq2d, xk2d, xv2d, wqkv):
    sq, sk = _col_scales()
    xin = np.stack([
        (xq2d.T * sq[None, :]).astype(NP_BF16),
        (xk2d.T * sk[None, :]).astype(NP_BF16),
        xv2d.T.astype(NP_BF16),
    ], axis=0)
    return {
        "xin": np.ascontiguousarray(xin),
        "wqkv": wqkv.astype(NP_BF16),
    }


def make_in_maps(inputs):
    """inputs: dict from setup_inputs (full batch). Returns per-core in_maps."""
    xq, xk, xv = inputs["xq"], inputs["xk"], inputs["xv"]
    ident = np.zeros((256, 128), np.float32)
    ident[0:128, 0:128] = np.eye(128, dtype=np.float32)
    wqkv = np.ascontiguousarray(np.concatenate(
        [np.asarray(inputs["Wq"], dtype=np.float32),
         np.asarray(inputs["Wk"], dtype=np.float32),
         np.asarray(inputs["Wv"], dtype=np.float32), ident], axis=1))
    in_maps = []
    for b in range(8):
        in_maps.append(prep_core_inputs(
            np.asarray(xq[b], dtype=np.float32),
            np.asarray(xk[b], dtype=np.float32),
            np.asarray(xv[b], dtype=np.float32), wqkv))
    return in_maps


_NC_CACHE = {}


def _get_nc():
    if "nc" not in _NC_CACHE:
        from concourse import bacc
        nc = bacc.Bacc("TRN2", target_bir_lowering=False, debug=False)
        build_kernel(nc)
        nc.compile()
        _NC_CACHE["nc"] = nc
    return _NC_CACHE["nc"]


def run(inputs, trace=False, **kwargs):
    """Run on 8 NeuronCores; returns (output [8,4000,256], BassKernelResults)."""
    from concourse.bass_utils import run_bass_kernel_spmd

    nc = _get_nc()
    in_maps = make_in_maps(inputs)
    res = run_bass_kernel_spmd(nc, in_maps, core_ids=list(range(8)),
                               trace=trace, **kwargs)
    out = np.stack([np.asarray(r["out"]).astype(np.float32)
                    for r in res.results], axis=0)
    return out, res


def kernel(**inputs) -> np.ndarray:
    out, _ = run(inputs)
    return out
